# revision 11
# baseline (speedup 1.0000x reference)
"""ContextAwareAttention Trainium2 Bass kernel (v2).

Per batch b (8 cores, one batch each; S=2048, D=1024, fp32 in/out):
    q = (query + context) @ Wq.T + bq   (context folded into bias on host)
    k = (key   + context) @ Wk.T + bk
    v = value @ Wv.T + bv
    scores = q @ k.T / sqrt(D), causal, softmax over keys
    out = softmax(scores) @ v

Design (v2, ~bf16 everywhere):
  * Data-parallel: batch b -> NeuronCore b (weights replicated).
  * All PE inputs bf16 (measured l2 rel err ~3e-3 vs f32 reference;
    gate is 2e-2). Host converts x/W to bf16: DMA volume halves and
    qT/kT/vT all fit in SBUF -- no DRAM scratch round trip.
  * Single fused pipeline over 512-seq chunks g=0..3:
        QK_g -> V_g -> A_g
    so the PE never drains between "phases"; attention dependency
    bubbles fill with projection GEMMs.
  * Scores are computed TRANSPOSED ([k 128, q 512] per key-block j,
    4 query tiles per group): exp writes P^T directly, eliminating all
    136 PE transposes and their PSUM->SBUF copies.
  * Softmax row-sums ride the PV accumulation as 1-row ones-matmuls
    reusing the PT_j stationary (osum PSUM [128,1]).
  * Epilogue: DVE reciprocal of osum, ACT scales PV output by it
    (per-partition scale), DVE adds the broadcast v-bias (bvb from
    host), DMA out in f32.
  * DMA queues: weights on scalar, x on sync, xv on gpsimd, consts +
    output on vector -- spreads sequencer cost, keeps startup prefix
    minimal (first matmul needs only wq[:, 0:256] slices + xq chunk 0).
"""

import os
import sys
import types

import numpy as np
import ml_dtypes

import concourse.bass as bass
import concourse.tile as tile
from concourse import bacc, mybir
from concourse.bass_utils import run_bass_kernel_spmd

F32 = mybir.dt.float32
BF16 = mybir.dt.bfloat16
F8 = mybir.dt.float8e4
DR = mybir.MatmulPerfMode.DoubleRow
AF = mybir.ActivationFunctionType

B, S, D = 8, 2048, 1024
NE = D // 128          # 8 feature chunks of the model dim on partitions
NST = S // 128         # 16 sequence tiles of 128
NG = S // 512          # 4 sequence chunks of 512
SCALE = float(D) ** -0.5
Q8 = 4.0                     # q/k fp8 pre-scale; folded out of exp
SCALE8 = SCALE / (Q8 * Q8)
N_CORES = 8
MASK_NEG = -1.0e30

LAST_EXEC_NS = None


def _install_ntff_hook():
    """Register the axon NTFF profiling hook (missing antenv.axon_hooks stub).
    Harmless no-op if anything is unavailable; only needed when BASS_TRACE=1."""
    try:
        if "antenv.axon_hooks" in sys.modules:
            return
        import antenv
        mod = types.ModuleType("antenv.axon_hooks")
        _hook = [None]
        mod.set_axon_ntff_profile_hook = lambda h: _hook.__setitem__(0, h)
        mod.get_axon_ntff_profile_hook = lambda: _hook[0]
        sys.modules["antenv.axon_hooks"] = mod
        antenv.axon_hooks = mod
        from trn_agent_boot.trn_boot import _ntff_profile_via_ctypes
        mod.set_axon_ntff_profile_hook(
            _ntff_profile_via_ctypes("/opt/axon/libaxon_pjrt.so"))
    except Exception:
        pass


def _build():
    nc = bacc.Bacc("TRN2", target_bir_lowering=False, debug=False,
                   num_devices=N_CORES)

    # x blocked [g, p, dp, c]; W blocked [p, dp, cols] (host pre-permuted)
    xqT = nc.dram_tensor("xqT", [NG, 128, NE, 512], BF16,
                         kind="ExternalInput").ap()
    xkT = nc.dram_tensor("xkT", [NG, 128, NE, 512], BF16,
                         kind="ExternalInput").ap()
    xvT = nc.dram_tensor("xvT", [NG, 128, NE, 512], BF16,
                         kind="ExternalInput").ap()
    # hot pack 0: wq cols 0:128 blocked + bqp + bkp + ones (first DMA);
    # hot pack 1: wq cols 128:512 blocked
    H0B = 2128
    hot0d = nc.dram_tensor("hot0d", [128, H0B], mybir.dt.uint8,
                           kind="ExternalInput").ap()
    hot1d = nc.dram_tensor("hot1d", [128, NE, 384], BF16,
                           kind="ExternalInput").ap()
    Wq2 = nc.dram_tensor("Wq2", [128, NE, 512], BF16,
                         kind="ExternalInput").ap()
    WkT = nc.dram_tensor("WkT", [128, NE, D], BF16,
                         kind="ExternalInput").ap()
    WvT = nc.dram_tensor("WvT", [128, NE, D], BF16, kind="ExternalInput").ap()
    # mask [128,128] f32 + bvb [128,1024] f32 packed
    MBB = 4608
    mbd = nc.dram_tensor("mbd", [128, MBB], mybir.dt.uint8,
                         kind="ExternalInput").ap()
    out_d = nc.dram_tensor("out", [S, D], F32, kind="ExternalOutput").ap()

    with tile.TileContext(nc) as tc:
        with tc.tile_pool(name="wp", bufs=1, side="left") as wp, \
             tc.tile_pool(name="kv", bufs=1, side="left") as kv, \
             tc.tile_pool(name="cst", bufs=1) as cp, \
             tc.tile_pool(name="xp", bufs=1) as xp, \
             tc.tile_pool(name="qp", bufs=1) as qp, \
             tc.tile_pool(name="ptp", bufs=1) as ptp, \
             tc.tile_pool(name="op", bufs=1) as op, \
             tc.tile_pool(name="ps", bufs=1, space="PSUM") as ps:

            # --- weight loads: blocked [128, dp, cols] tiles, two issues
            # per tensor (e-prefix first so the first groups start early).
            # Keeping the scalar queue nearly DMA-free is critical: each
            # dma_start costs ~600ns of sequencer time, and the QK/V PSUM
            # evacuations share that queue.
            # Everything startup-critical rides the SP queue (it gets the
            # widest DMA-engine share); bulk (wk/wv/xv/mask/bvb) rides the
            # Pool queue. The ACT queue carries no DMAs at all. Small
            # consts are packed into the hot wq transfer: a separate
            # [128, tiny] DMA costs 128 descriptors of queue time.
            hot0t = wp.tile([128, H0B], mybir.dt.uint8, tag="hot0",
                            name="hot0")
            nc.sync.dma_start(hot0t[:], hot0d)
            hot0w = hot0t[:, 0:2048].bitcast(BF16)    # [128, 1024] = e0 blk
            bqpt = hot0t[:, 2048:2080].bitcast(F32)   # [128, 8]
            bkpt = hot0t[:, 2080:2112].bitcast(F32)   # [128, 8]
            onet = hot0t[:, 2112:2114].bitcast(BF16)  # [128, 1]
            hot1t = wp.tile([128, NE, 384], BF16, tag="hot1", name="hot1")
            wq2t = wp.tile([128, NE, 512], BF16, tag="wq2", name="wq2")
            wkt = wp.tile([128, NE, D], BF16, tag="wk", name="wk")
            wvt = wp.tile([128, NE, D], BF16, tag="wv", name="wv")
            mbt = wp.tile([128, MBB], mybir.dt.uint8, tag="mb", name="mb")
            mskt = mbt[:, 0:512].bitcast(F32)         # [128, 128]
            bvbt = mbt[:, 512:4608].bitcast(F32)      # [128, 1024]

            # --- SBUF residents: kT [e][128, S], v [j][128, D], all bf16
            kres = [kv.tile([128, 2, S], F8, tag=f"k{p}", name=f"kres{p}")
                    for p in range(NE // 2)]
            vres = [kv.tile([128, D], BF16, tag=f"v{j}", name=f"vres{j}")
                    for j in range(NST)]

            def load_x(g, which, src, eng, split=False):
                a = xp.tile([128, NE, 512], BF16, tag=f"x{which}", bufs=1,
                            name=f"x{which}")
                if split:
                    nc.sync.dma_start(a[:, 0:4, :], src[g, :, 0:4, :])
                    nc.gpsimd.dma_start(a[:, 4:NE, :], src[g, :, 4:NE, :])
                else:
                    eng.dma_start(a[:], src[g])
                return a

            def q_group(e, xqb, qch):
                psq = ps.tile([128, 512], F32, tag="pj", bufs=3, name="psq")
                for dp in range(NE):
                    if e == 0:
                        wsl = hot0w[:, dp * 128:(dp + 1) * 128]
                    elif e < 4:
                        wsl = hot1t[:, dp, (e - 1) * 128:e * 128]
                    else:
                        wsl = wq2t[:, dp, (e - 4) * 128:(e - 3) * 128]
                    nc.tensor.matmul(psq[:], wsl,
                                     xqb[:, dp, :], start=(dp == 0),
                                     stop=(dp == NE - 1))
                nc.scalar.activation(qch[e // 2][:, e % 2, :], psq[:],
                                     AF.Identity, scale=Q8,
                                     bias=bqpt[:, e:e + 1])

            def k_group(g, e, xkb):
                psk = ps.tile([128, 512], F32, tag="pj", bufs=3, name="psk")
                for dp in range(NE):
                    nc.tensor.matmul(psk[:], wkt[:, dp, e * 128:(e + 1) * 128],
                                     xkb[:, dp, :], start=(dp == 0),
                                     stop=(dp == NE - 1))
                nc.scalar.activation(
                    kres[e // 2][:, e % 2, g * 512:(g + 1) * 512], psk[:],
                    AF.Identity, scale=Q8, bias=bkpt[:, e:e + 1])

            def v_group(g, dc, s4, xvb):
                j = g * 4 + s4
                psv = ps.tile([128, 512], F32, tag="pj", bufs=3, name="psv")
                for dp in range(NE):
                    nc.tensor.matmul(
                        psv[:], xvb[:, dp, s4 * 128:(s4 + 1) * 128],
                        wvt[:, dp, dc * 512:(dc + 1) * 512],
                        start=(dp == 0), stop=(dp == NE - 1))
                nc.scalar.copy(vres[j][:, dc * 512:(dc + 1) * 512], psv[:])

            def attn_group(g, qch):
                nj = 4 * g + 4
                # scores (transposed [k, q]) + exp -> PT_j, per key block j.
                # Diagonal key blocks (cj = j - 4g >= 0) only compute the
                # unmasked q-window [cj*128, 512); the cq == cj subtile gets
                # the triangular mask, earlier subtiles are never read.
                # Narrow diagonal matmuls are interleaved with wide ones so
                # their LDWEIGHTS hide under the wide matmuls' streaming.
                pts = [None] * nj
                for j in range(nj):
                    pts[j] = ptp.tile([128, 512], BF16, tag=f"pt{j}", bufs=1,
                                      name=f"pt{j}")

                def score_block(j):
                    cj = j - 4 * g
                    qoff = max(cj, 0) * 128
                    w = 512 - qoff
                    pss = ps.tile([128, 512], F32, tag="sc", bufs=2,
                                  name="pss")
                    for p in range(NE // 2):
                        nc.tensor.matmul(
                            pss[:, 0:w],
                            kres[p][:, :, j * 128:(j + 1) * 128],
                            qch[p][:, :, qoff:512], start=(p == 0),
                            stop=(p == NE // 2 - 1), perf_mode=DR)
                    if cj >= 0:
                        nc.vector.tensor_add(pss[:, 0:128], pss[:, 0:128],
                                             mskt[:])
                    nc.scalar.activation(pts[j][:, qoff:512], pss[:, 0:w],
                                         AF.Exp, scale=SCALE8)

                def score_pair(ja, jb):
                    # interleave a wide and a narrow block e-by-e on two
                    # psum tiles so every LDWEIGHTS hides under streaming
                    cja, cjb = ja - 4 * g, jb - 4 * g
                    qa, qb = max(cja, 0) * 128, max(cjb, 0) * 128
                    wa, wb = 512 - qa, 512 - qb
                    pa_ = ps.tile([128, 512], F32, tag="sc", bufs=2,
                                  name="pssa")
                    pb_ = ps.tile([128, 512], F32, tag="sc", bufs=2,
                                  name="pssb")
                    for p in range(NE // 2):
                        nc.tensor.matmul(
                            pa_[:, 0:wa],
                            kres[p][:, :, ja * 128:(ja + 1) * 128],
                            qch[p][:, :, qa:512], start=(p == 0),
                            stop=(p == NE // 2 - 1), perf_mode=DR)
                        nc.tensor.matmul(
                            pb_[:, 0:wb],
                            kres[p][:, :, jb * 128:(jb + 1) * 128],
                            qch[p][:, :, qb:512], start=(p == 0),
                            stop=(p == NE // 2 - 1), perf_mode=DR)
                    for (j, cj, qoff, w, pp) in ((ja, cja, qa, wa, pa_),
                                                 (jb, cjb, qb, wb, pb_)):
                        if cj >= 0:
                            nc.vector.tensor_add(pp[:, 0:128], pp[:, 0:128],
                                                 mskt[:])
                        nc.scalar.activation(pts[j][:, qoff:512], pp[:, 0:w],
                                             AF.Exp, scale=SCALE8)

                for j in range(4 * g):
                    score_block(j)
                score_pair(4 * g + 0, 4 * g + 3)
                score_pair(4 * g + 1, 4 * g + 2)
                # PV + rowsum + epilogue, per query tile in the group
                for cq in range(4):
                    t = 4 * g + cq
                    o0 = ps.tile([128, 512], F32, tag="o0", bufs=1, name="o0")
                    o1 = ps.tile([128, 512], F32, tag="o1", bufs=1, name="o1")
                    osum = ps.tile([128, 1], F32, tag="os", bufs=1, name="os")
                    for j in range(t + 1):
                        pj = pts[j][:, cq * 128:(cq + 1) * 128]
                        st = (j == 0)
                        sp = (j == t)
                        nc.tensor.matmul(o0[:], pj, vres[j][:, 0:512],
                                         start=st, stop=sp)
                        nc.tensor.matmul(o1[:], pj, vres[j][:, 512:1024],
                                         start=st, stop=sp)
                        nc.tensor.matmul(osum[:], pj, onet[:],
                                         start=st, stop=sp)
                    rcp = op.tile([128, 1], F32, tag="rcp", bufs=2, name="rcp")
                    nc.vector.reciprocal(rcp[:], osum[:])
                    ot = op.tile([128, D], F32, tag="ot", bufs=2, name="ot")
                    for dc in range(2):
                        dsl = slice(dc * 512, (dc + 1) * 512)
                        eng = (nc.sync, nc.gpsimd)[(t + dc) % 2]
                        nc.scalar.activation(ot[:, dsl],
                                             (o0 if dc == 0 else o1)[:],
                                             AF.Copy, scale=rcp[:])
                        nc.vector.tensor_add(ot[:, dsl], ot[:, dsl],
                                             bvbt[:, dsl])
                        eng.dma_start(out_d[t * 128:(t + 1) * 128, dsl],
                                      ot[:, dsl])

            for g in range(NG):
                # g=0: xq rides the Pool queue so it transfers concurrently
                # with the hot pack on SP; later chunks ride SP.
                xqb = load_x(g, "q", xqT, nc.sync, split=(g == 0))
                if g == 0:
                    nc.sync.dma_start(hot1t[:], hot1d)
                    nc.sync.dma_start(wq2t[:], Wq2)
                xkb = load_x(g, "k", xkT, nc.sync)
                xvb = load_x(g, "v", xvT, nc.gpsimd)
                if g == 0:
                    nc.gpsimd.dma_start(wkt[:], WkT)
                    nc.gpsimd.dma_start(wvt[:], WvT)
                    nc.gpsimd.dma_start(mbt[:], mbd)
                qch = [qp.tile([128, 2, 512], F8, tag=f"q{p}", bufs=2,
                               name=f"qch{p}") for p in range(NE // 2)]
                for e in range(NE):
                    q_group(e, xqb, qch)
                for e in range(NE):
                    k_group(g, e, xkb)
                for dc in range(2):
                    for s4 in range(4):
                        v_group(g, dc, s4, xvb)
                attn_group(g, qch)

    nc.compile()
    return nc


_NC = [None]


def kernel(query, key, value, context, Wq, bq, Wk, bk, Wv, bv):
    global LAST_EXEC_NS
    f32 = np.float32
    bf16 = ml_dtypes.bfloat16
    query = np.asarray(query, f32)
    key = np.asarray(key, f32)
    value = np.asarray(value, f32)
    context = np.asarray(context, f32)
    Wq = np.asarray(Wq, f32)
    bq = np.asarray(bq, f32)
    Wk = np.asarray(Wk, f32)
    bk = np.asarray(bk, f32)
    Wv = np.asarray(Wv, f32)
    bv = np.asarray(bv, f32)

    if _NC[0] is None:
        _NC[0] = _build()
    nc = _NC[0]

    # context folded into effective q/k biases (exact)
    bq_eff = bq + Wq @ context
    bk_eff = bk + Wk @ context
    # [128, 8]: bias for e-chunk e in column e, partition = within-chunk idx
    bqp = np.ascontiguousarray(bq_eff.reshape(NE, 128).T)
    bkp = np.ascontiguousarray(bk_eff.reshape(NE, 128).T)
    bvb = np.ascontiguousarray(np.broadcast_to(bv, (128, D))).astype(f32)
    def wblk(W):
        # W.T [d, e] -> [p, dp, e-cols], p-major contiguous
        return np.ascontiguousarray(
            W.T.astype(bf16).reshape(NE, 128, D).transpose(1, 0, 2))
    WqB = wblk(Wq)
    WkT = wblk(Wk)
    WvT = wblk(Wv)
    Wq2 = np.ascontiguousarray(WqB[:, :, 512:])
    hot1d = np.ascontiguousarray(WqB[:, :, 128:512])
    # hot pack 0: wq col-block e0 + bqp + bkp + ones(bf16)
    hot0d = np.zeros((128, 2128), np.uint8)
    hot0d[:, 0:2048] = np.ascontiguousarray(
        WqB[:, :, 0:128]).view(np.uint8).reshape(128, 2048)
    hot0d[:, 2048:2080] = (bqp * Q8).view(np.uint8)
    hot0d[:, 2080:2112] = (bkp * Q8).view(np.uint8)
    hot0d[:, 2112:2114] = np.ones((128, 1), bf16).view(np.uint8)
    # mask + bvb pack
    mskg = np.tril(np.full((128, 128), MASK_NEG, f32), -1)
    mbd = np.zeros((128, 4608), np.uint8)
    mbd[:, 0:512] = mskg.view(np.uint8)
    mbd[:, 512:4608] = bvb.view(np.uint8)

    def xblk(x):
        # x [s, d] -> x.T [d, s] -> [g, p, dp, c], contiguous per partition
        return np.ascontiguousarray(
            x.T.astype(bf16).reshape(NE, 128, NG, 512).transpose(2, 1, 0, 3))
    in_maps = []
    for b in range(B):
        in_maps.append({
            "xqT": xblk(query[b]),
            "xkT": xblk(key[b]),
            "xvT": xblk(value[b]),
            "hot0d": hot0d, "hot1d": hot1d, "Wq2": Wq2, "WkT": WkT, "WvT": WvT,
            "mbd": mbd,
        })

    trace = bool(os.environ.get("BASS_TRACE"))
    if trace:
        _install_ntff_hook()
    res = run_bass_kernel_spmd(nc, in_maps, list(range(N_CORES)), trace=trace)
    LAST_EXEC_NS = res.exec_time_ns
    return np.stack([res.results[b]["out"] for b in range(B)], axis=0)
